# revision 18
# baseline (speedup 1.0000x reference)
"""Trainium2 Bass kernel for the CGA sandwich pipeline (nn_CGAPipeline).

out = decode( (V * encode(x)) * ~V ) over N=2^21 points, data-parallel over
8 NeuronCores.

v3 design ("POP" = point-on-partition layout, multi-engine roofline):

The v2 comp-major design was bound by PSUM-evacuation copies (ACT), 1x-mode
DVE products reading f32 PSUM, and slow gpsimd adds; all four engines sat at
50-80% of a 484us span.  v3 keeps every per-point tensor in a point-major
"comp-blocked" SBUF layout [128 part = point-rows, free = j*128 + q] where
j = odd-blade rank (16) and q = point-in-row (128):

- The five XOR-translation permutations j -> j^c of the versor become pure
  access patterns (multi-dim APs with negative strides), zero compute.
- The Clifford sign cocycle is split as s(j,p) = sigma(j)*tau(j^c)*chi(j):
  tau is folded into the host-shipped versor copy, sigma into the stage-2
  tree weights, and the residual characters chi into sign-alternating
  broadcast buffers (stage 1) and +-identity matmul weights (stage 2).
- Stage-1/stage-2 products are bf16 tensor_tensor ops in DVE 2x_1p mode
  (all-SBUF, unit innermost stride), split 8/2 between DVE and GpSimd.
- The j-sums (stage-1 term accumulation and stage-2 contraction trees) run
  on the otherwise-idle PE as +-identity matmuls accumulating in PSUM f32.
- ACT only evacuates mx and the 4 output channels; decode division and the
  ill-conditioned-point fixup stay on the host as in v2.
"""
import sys

sys.path.insert(0, "/opt/trn_rl_repo")

import ml_dtypes
import numpy as np

import concourse.bacc as bacc
import concourse.bass as bass
import concourse.mybir as mybir
import concourse.tile as tile
from concourse.bass_types import AP
from concourse.bass_utils import run_bass_kernel_spmd

F32 = mybir.dt.float32
BF16 = mybir.dt.bfloat16

# ----------------------------------------------------------------------------
# Cl(4,1) sign tables (rank-indexed; see reference.py for the blade algebra)
# ----------------------------------------------------------------------------
_METRIC = [1.0, 1.0, 1.0, 1.0, -1.0]


def _popcount(x):
    return bin(x).count("1")


def _blade_mul(a, b):
    s = 0
    t = a >> 1
    while t:
        s += _popcount(t & b)
        t >>= 1
    sign = -1.0 if (s & 1) else 1.0
    for i in range(5):
        if (a >> i) & 1 and (b >> i) & 1:
            sign *= _METRIC[i]
    return a ^ b, sign


def _rev_sign(b):
    g = _popcount(b)
    return -1.0 if (g * (g - 1) // 2) % 2 else 1.0


def _E_code(i):
    return (i << 1) | (_popcount(i) & 1)


def _O_code(j):
    return (j << 1) | ((_popcount(j) + 1) & 1)


_KAPPAS = [1, 2, 4, 8, 16]
CS = [k >> 1 for k in _KAPPAS]  # XOR-translation constants [0,1,2,4,8]
J16 = np.arange(16)

_s1 = np.zeros((16, 5), np.float64)
_s2 = np.zeros((16, 5), np.float64)
for _p, _kp in enumerate(_KAPPAS):
    _c = _kp >> 1
    for _j in range(16):
        _code, _sg = _blade_mul(_E_code(_j ^ _c), _kp)
        assert _code == _O_code(_j)
        _s1[_j, _p] = _sg
for _q, _kq in enumerate(_KAPPAS):
    _c = _kq >> 1
    for _j in range(16):
        _code, _sg = _blade_mul(_O_code(_j), _E_code(_j ^ _c))
        assert _code == _kq
        _s2[_j, _q] = _sg * _rev_sign(_E_code(_j ^ _c))

# Sign separation: s1[j,p] = SIGMA[j]*TAU[j^c_p]*EPS1[j,p] with EPS1 a GF(2)
# character per column; s2[j,q]*SIGMA[j]*TAU[j^c_q] = W2[j,q] goes into the
# stage-2 tree weights.  (sigma/tau found by exhaustive search.)
SIGMA = np.array([-1, 1, 1, 1, 1, 1, -1, 1, 1, 1, -1, 1, -1, 1, 1, 1], np.float64)
TAU = np.array([1, 1, -1, 1, -1, 1, 1, 1, 1, -1, -1, -1, -1, -1, 1, -1], np.float64)

EPS1 = np.stack([SIGMA * _s1[:, p] * TAU[J16 ^ CS[p]] for p in range(5)], axis=1)
W2 = np.stack([_s2[:, q] * SIGMA * TAU[J16 ^ CS[q]] for q in range(5)], axis=1)

# stage-1 residual characters: support of chi per channel, verified below
#   p=0: chi_6 base -1 (3-slot alternating buffer over j1+j2)
#   p=1: chi_9 base -1 (3 slots over j0+j3)
#   p=2: chi_4 base +1 (2 slots over j2)
#   p=3,4: constant +1 (1 slot)
for _p, (_a, _e) in enumerate([(6, -1.0), (9, -1.0), (4, 1.0), (0, 1.0), (0, 1.0)]):
    for _j in range(16):
        assert EPS1[_j, _p] == _e * ((-1.0) ** _popcount(_a & _j)), (
            f"EPS1 char mismatch p={_p}"
        )

# ----------------------------------------------------------------------------
# Geometry
# ----------------------------------------------------------------------------
N_TOTAL = 2097152
N_CORES = 8
NPC = N_TOTAL // N_CORES  # 262144 points per core
B = 128                   # points per j-block (free-dim inner run)
NJ = 16
MACRO = 128 * B           # 16384 points per macro tile
NM = NPC // MACRO         # 16 macros per core
UW = NJ * B               # 2048 u columns per macro
XTW = 50 * B              # xt blocks: grid-p0[16] grid-p1[16] grid-p2[16] hm hp
OW = 4 * B                # out channels: o0 o1 o2 s

WEIGHTS = {
    "wident": np.concatenate(
        [np.eye(128, dtype=np.float32), -np.eye(128, dtype=np.float32)], axis=1
    ).astype(ml_dtypes.bfloat16)
}


def _ap(t_ap, off, dims):
    """Custom free-dim AP on a tile: keep partition dim, replace free dims."""
    p = t_ap.ap[0]
    return AP(t_ap.tensor, t_ap.offset + off, [list(p)] + [list(d) for d in dims])


def _perm_ap(u_ap, c):
    """AP reading u[:, (j^c)*B + q] in plain (j,q) iteration order.
    Unflipped low j-bits merge into the innermost run, keeping every AP
    within the TENSOR3D 3-free-dim ISA limit."""
    if c == 0:
        return _ap(u_ap, 0, [[1, UW]])
    if c == 1:
        return _ap(u_ap, B, [[2 * B, 8], [-B, 2], [1, B]])
    if c == 2:
        return _ap(u_ap, 2 * B, [[4 * B, 4], [-2 * B, 2], [1, 2 * B]])
    if c == 4:
        return _ap(u_ap, 4 * B, [[8 * B, 2], [-4 * B, 2], [1, 4 * B]])
    if c == 8:
        return _ap(u_ap, 8 * B, [[-8 * B, 2], [1, 8 * B]])
    raise ValueError(c)


def build_bass():
    nc = bacc.Bacc("TRN2")

    u_d = nc.dram_tensor("u", [NM, 128, UW], BF16, kind="ExternalInput")
    xt_d = nc.dram_tensor("xt", [NM, 128, XTW], BF16, kind="ExternalInput")
    o_d = nc.dram_tensor("out", [NM, 128, OW], F32, kind="ExternalOutput")
    w_d = nc.dram_tensor("wident", [128, 256], BF16, kind="ExternalInput")

    from contextlib import ExitStack

    with tile.TileContext(nc) as tc, ExitStack() as ctx:
        wpool = ctx.enter_context(tc.tile_pool(name="wpool", bufs=1))
        w_sb = wpool.tile([128, 256], BF16, tag="wident")
        nc.sync.dma_start(w_sb[:], w_d[:])
        IP = w_sb[:, 0:128]   # +identity
        IN = w_sb[:, 128:256]  # -identity

        io_u = ctx.enter_context(tc.tile_pool(name="io_u", bufs=4))
        io_x = ctx.enter_context(tc.tile_pool(name="io_x", bufs=3))
        tp = ctx.enter_context(tc.tile_pool(name="tp", bufs=2))
        mxp = ctx.enter_context(tc.tile_pool(name="mxp", bufs=2))
        zp = ctx.enter_context(tc.tile_pool(name="zp", bufs=2))
        op = ctx.enter_context(tc.tile_pool(name="op", bufs=2))
        ps_mx = ctx.enter_context(tc.tile_pool(name="ps_mx", bufs=1, space="PSUM"))
        ps_o = ctx.enter_context(tc.tile_pool(name="ps_o", bufs=2, space="PSUM"))

        # per-macro state carried across the software pipeline
        state = {}  # m -> dict(u=..., mx=..., z=[...])

        def emit_fetch(m):
            u = io_u.tile([128, UW], BF16, tag="u")
            nc.sync.dma_start(u[:], u_d[m])
            xt = io_x.tile([128, XTW], BF16, tag="xt")
            nc.sync.dma_start(xt[:], xt_d[m])
            state[m] = {"u": u, "xt": xt}

        def emit_front(m, prev):
            """Stage-1 products (interleaved with prev's stage-2 DVE
            products) + PE accumulation + mx evac."""
            u, xt = state[m]["u"], state[m]["xt"]

            if prev is not None:
                emit_mid_pool(prev)

            # stage-1 products on DVE: one op per channel.  p0-p2 read
            # full-grid sign-expanded x buffers (16 blocks, content
            # EPS1[j,p]*x_p); hm/hp are plain stride-0 broadcasts.
            ts = []
            spec = [
                (0, _ap(xt[:], 0, [[B, 16], [1, B]])),            # p0: c=0
                (1, _ap(xt[:], 16 * B, [[B, 16], [1, B]])),       # p1: c=1
                (2, _ap(xt[:], 32 * B, [[B, 16], [1, B]])),       # p2: c=2
                (4, _ap(xt[:], 48 * B, [[0, 16], [1, B]])),       # p3: c=4 (hm)
                (8, _ap(xt[:], 49 * B, [[0, 16], [1, B]])),       # p4: c=8 (hp)
            ]
            for i, (c, bc) in enumerate(spec):
                t = tp.tile([128, UW], BF16, tag=f"t{i}")
                nc.vector.tensor_mul(_ap(t[:], 0, [[1, UW]]), _perm_ap(u[:], c), bc)
                ts.append(t)
                # spread prev's stage-2 DVE products between stage-1 products
                if prev is not None and i in (1, 2, 3):
                    emit_mid_dve(prev, q=i)

            # PE: accumulate the five t tiles into PSUM f32 (p-major so the
            # accumulation chases the DVE product stream)
            mx_ps = ps_mx.tile([128, UW], F32, tag="mx_ps")
            for p in range(5):
                for b in range(4):
                    sl = slice(b * 512, (b + 1) * 512)
                    nc.tensor.matmul(
                        mx_ps[:, sl], IP, ts[p][:, sl], start=(p == 0), stop=(p == 4)
                    )

            mx = mxp.tile([128, UW], BF16, tag="mx")
            nc.scalar.copy(mx[:], mx_ps[:])
            state[m]["mx"] = mx

        def emit_mid_pool(m):
            """Stage-2 products on Pool (z4 first: the s-tree consumes it)."""
            st = state[m]
            u, mx = st["u"], st["mx"]
            st["z"] = {}
            for q in (4, 0):
                z = zp.tile([128, UW], BF16, tag=f"z{q}")
                nc.gpsimd.tensor_mul(z[:], _perm_ap(u[:], CS[q]), mx[:])
                st["z"][q] = z

        def emit_mid_dve(m, q):
            """One stage-2 product on DVE (permuted AP in src0 — src1 with a
            short-run multi-dim AP drops the DVE out of 2x mode)."""
            st = state[m]
            z = zp.tile([128, UW], BF16, tag=f"z{q}")
            nc.vector.tensor_mul(z[:], _perm_ap(st["u"][:], CS[q]), st["mx"][:])
            st["z"][q] = z

        def emit_back(m):
            """PE contraction trees + out evac + DMA out."""
            zs = state[m]["z"]
            o_ps = ps_o.tile([128, OW], F32, tag="o_ps")
            # channels o1,o2 first (their z's finish earliest on DVE), then
            # s (z4 is Pool's first product), then o0 (z0 is Pool's second)
            for q in (1, 2):
                for j in range(16):
                    wgt = IP if W2[j, q] > 0 else IN
                    nc.tensor.matmul(
                        o_ps[:, q * B:(q + 1) * B],
                        wgt,
                        zs[q][:, j * B:(j + 1) * B],
                        start=(j == 0),
                        stop=(j == 15),
                    )
            # channel s = sum_j W2[j,4]*z4[j]  -  sum_j W2[j,3]*z3[j]
            for k, (q, flip) in enumerate(((4, 1.0), (3, -1.0))):
                for j in range(16):
                    wgt = IP if flip * W2[j, q] > 0 else IN
                    nc.tensor.matmul(
                        o_ps[:, 3 * B:4 * B],
                        wgt,
                        zs[q][:, j * B:(j + 1) * B],
                        start=(k == 0 and j == 0),
                        stop=(k == 1 and j == 15),
                    )
            for j in range(16):
                wgt = IP if W2[j, 0] > 0 else IN
                nc.tensor.matmul(
                    o_ps[:, 0:B],
                    wgt,
                    zs[0][:, j * B:(j + 1) * B],
                    start=(j == 0),
                    stop=(j == 15),
                )
            o_sb = op.tile([128, OW], F32, tag="o_sb")
            nc.scalar.copy(o_sb[:], o_ps[:])
            nc.sync.dma_start(o_d[m], o_sb[:])
            del state[m]

        # software pipeline: fetch(m+2) | front(m) [z(m-1) interleaved] |
        # back(m-1)
        emit_fetch(0)
        emit_fetch(1)
        emit_front(0, None)
        for m in range(1, NM):
            emit_fetch(m + 1) if m + 1 < NM else None
            emit_front(m, m - 1)
            emit_back(m - 1)
        emit_mid_pool(NM - 1)
        for q in (1, 2, 3):
            emit_mid_dve(NM - 1, q)
        emit_back(NM - 1)

    nc.compile()
    return nc


_NC_CACHE = None


def _get_nc():
    global _NC_CACHE
    if _NC_CACHE is None:
        _NC_CACHE = build_bass()
    return _NC_CACHE


def _host_prep(versor, x):
    """Build the per-core input tensors (pure layout/sign/dtype transforms)."""
    # u[m,p,j*B+q] = TAU[j] * versor[n, j],  n = m*MACRO + p*B + q
    v5 = versor.reshape(N_CORES, NM, 128, B, 16)
    u = np.ascontiguousarray(
        np.transpose(v5, (0, 1, 2, 4, 3)) * TAU[None, None, None, :, None]
    ).astype(ml_dtypes.bfloat16)
    u = u.reshape(N_CORES, NM, 128, UW)

    xf = x.astype(np.float64)
    h = 0.5 * np.einsum("ij,ij->i", xf, xf)
    # 50 blocks: three full-grid channels (EPS1[j,p] * x_p for all 16 j),
    # then the two constant-sign h channels.
    blocks = [EPS1[j, p] * xf[:, p] for p in range(3) for j in range(16)]
    blocks += [h - 0.5, h + 0.5]
    xt = np.stack(blocks, axis=1)  # [N, 50]
    xt = xt.reshape(N_CORES, NM, 128, B, 50)
    xt = np.ascontiguousarray(np.transpose(xt, (0, 1, 2, 4, 3))).astype(
        ml_dtypes.bfloat16
    )
    xt = xt.reshape(N_CORES, NM, 128, XTW)
    return u, xt


def _in_maps(versor, x):
    u, xt = _host_prep(versor, x)
    in_maps = []
    for c in range(N_CORES):
        im = {"u": u[c], "xt": xt[c]}
        for name, arr in WEIGHTS.items():
            im[name] = arr
        in_maps.append(im)
    return in_maps


def _assemble(res):
    """Device [NM, 128, 4*B] channel tiles -> (N, 4) [num0,num1,num2,s]."""
    per_core = []
    for c in range(N_CORES):
        o = res.results[c]["out"].astype(np.float32).reshape(NM, 128, 4, B)
        per_core.append(np.transpose(o, (0, 1, 3, 2)).reshape(NPC, 4))
    return np.concatenate(per_core, axis=0)


def kernel(versor: np.ndarray, x: np.ndarray) -> np.ndarray:
    versor = np.ascontiguousarray(versor, dtype=np.float32)
    x = np.ascontiguousarray(x, dtype=np.float32)
    nc = _get_nc()
    res = run_bass_kernel_spmd(nc, _in_maps(versor, x), core_ids=list(range(N_CORES)))
    out4 = _assemble(res)
    num = out4[:, :3]
    sk = out4[:, 3]
    out = num / sk[:, None]

    # Conditioning fixup: bf16 on-chip products round at ~2^-9; points with a
    # small denominator s or large h amplify that beyond the error budget.
    # Recompute those few points exactly on the host.
    h = 0.5 * np.einsum("ij,ij->i", x, x)
    flag = (np.abs(sk) < 0.7) | (h > 4.5) | (np.abs(num).max(axis=1) > 4.0)
    if np.any(flag):
        out[flag] = _exact_ref(versor[flag], x[flag])
    return out.astype(np.float32)


def _exact_ref(versor, x):
    v = versor.astype(np.float64)
    xf = x.astype(np.float64)
    h = 0.5 * np.sum(xf * xf, axis=1)

    def X(c):
        return v[:, np.arange(16) ^ c]

    T0 = X(0) * (_s1[None, :, 0] * xf[:, 0:1])
    T1 = X(1) * (_s1[None, :, 1] * xf[:, 1:2])
    T2 = X(2) * (_s1[None, :, 2] * xf[:, 2:3])
    Vinf = _s1[None, :, 3] * X(4) + _s1[None, :, 4] * X(8)
    Cp = -0.5 * _s1[None, :, 3] * X(4) + 0.5 * _s1[None, :, 4] * X(8)
    mx = T0 + T1 + T2 + Vinf * h[:, None] + Cp
    D = _s2[None, :, 4] * X(8) - _s2[None, :, 3] * X(4)
    s = np.sum(mx * D, axis=1)
    num = np.stack(
        [np.sum(_s2[None, :, r] * (mx * X(r)), axis=1) for r in range(3)], axis=1
    )
    return (num / s[:, None]).astype(np.float32)


if __name__ == "__main__":
    rng = np.random.default_rng(0)
    v = (0.1 * rng.standard_normal((N_TOTAL, 16))).astype(np.float32)
    v[:, 0] += 1.0
    x = rng.standard_normal((N_TOTAL, 3)).astype(np.float32)
    out = kernel(versor=v, x=x)
    print("kernel ran, out shape", out.shape, out.dtype)


# revision 22
# speedup vs baseline: 1.5898x; 1.5898x over previous
"""Trainium2 Bass kernel for the CGA sandwich pipeline (nn_CGAPipeline).

out = decode( (V * encode(x)) * ~V ) over N=2^21 points, data-parallel over
8 NeuronCores.

v3 design ("POP" = point-on-partition layout, multi-engine roofline):

The v2 comp-major design was bound by PSUM-evacuation copies (ACT), 1x-mode
DVE products reading f32 PSUM, and slow gpsimd adds; all four engines sat at
50-80% of a 484us span.  v3 keeps every per-point tensor in a point-major
"comp-blocked" SBUF layout [128 part = point-rows, free = j*128 + q] where
j = odd-blade rank (16) and q = point-in-row (128):

- The five XOR-translation permutations j -> j^c of the versor become pure
  access patterns (multi-dim APs with negative strides), zero compute.
- The Clifford sign cocycle is split as s(j,p) = sigma(j)*tau(j^c)*chi(j):
  tau is folded into the host-shipped versor copy, sigma into the stage-2
  tree weights, and the residual characters chi into sign-alternating
  broadcast buffers (stage 1) and +-identity matmul weights (stage 2).
- Stage-1/stage-2 products are bf16 tensor_tensor ops in DVE 2x_1p mode
  (all-SBUF, unit innermost stride), split 8/2 between DVE and GpSimd.
- The j-sums (stage-1 term accumulation and stage-2 contraction trees) run
  on the otherwise-idle PE as +-identity matmuls accumulating in PSUM f32.
- ACT only evacuates mx and the 4 output channels; decode division and the
  ill-conditioned-point fixup stay on the host as in v2.
"""
import sys

sys.path.insert(0, "/opt/trn_rl_repo")

import ml_dtypes
import numpy as np

import concourse.bacc as bacc
import concourse.bass as bass
import concourse.mybir as mybir
import concourse.tile as tile
from concourse.bass_types import AP
from concourse.bass_utils import run_bass_kernel_spmd

F32 = mybir.dt.float32
BF16 = mybir.dt.bfloat16

# ----------------------------------------------------------------------------
# Cl(4,1) sign tables (rank-indexed; see reference.py for the blade algebra)
# ----------------------------------------------------------------------------
_METRIC = [1.0, 1.0, 1.0, 1.0, -1.0]


def _popcount(x):
    return bin(x).count("1")


def _blade_mul(a, b):
    s = 0
    t = a >> 1
    while t:
        s += _popcount(t & b)
        t >>= 1
    sign = -1.0 if (s & 1) else 1.0
    for i in range(5):
        if (a >> i) & 1 and (b >> i) & 1:
            sign *= _METRIC[i]
    return a ^ b, sign


def _rev_sign(b):
    g = _popcount(b)
    return -1.0 if (g * (g - 1) // 2) % 2 else 1.0


def _E_code(i):
    return (i << 1) | (_popcount(i) & 1)


def _O_code(j):
    return (j << 1) | ((_popcount(j) + 1) & 1)


_KAPPAS = [1, 2, 4, 8, 16]
CS = [k >> 1 for k in _KAPPAS]  # XOR-translation constants [0,1,2,4,8]
J16 = np.arange(16)

_s1 = np.zeros((16, 5), np.float64)
_s2 = np.zeros((16, 5), np.float64)
for _p, _kp in enumerate(_KAPPAS):
    _c = _kp >> 1
    for _j in range(16):
        _code, _sg = _blade_mul(_E_code(_j ^ _c), _kp)
        assert _code == _O_code(_j)
        _s1[_j, _p] = _sg
for _q, _kq in enumerate(_KAPPAS):
    _c = _kq >> 1
    for _j in range(16):
        _code, _sg = _blade_mul(_O_code(_j), _E_code(_j ^ _c))
        assert _code == _kq
        _s2[_j, _q] = _sg * _rev_sign(_E_code(_j ^ _c))

# Sign separation: s1[j,p] = SIGMA[j]*TAU[j^c_p]*EPS1[j,p] with EPS1 a GF(2)
# character per column; s2[j,q]*SIGMA[j]*TAU[j^c_q] = W2[j,q] goes into the
# stage-2 tree weights.  (sigma/tau found by exhaustive search.)
SIGMA = np.array([-1, 1, 1, 1, 1, 1, -1, 1, 1, 1, -1, 1, -1, 1, 1, 1], np.float64)
TAU = np.array([1, 1, -1, 1, -1, 1, 1, 1, 1, -1, -1, -1, -1, -1, 1, -1], np.float64)

EPS1 = np.stack([SIGMA * _s1[:, p] * TAU[J16 ^ CS[p]] for p in range(5)], axis=1)
W2 = np.stack([_s2[:, q] * SIGMA * TAU[J16 ^ CS[q]] for q in range(5)], axis=1)

# stage-1 residual characters: support of chi per channel, verified below
#   p=0: chi_6 base -1 (3-slot alternating buffer over j1+j2)
#   p=1: chi_9 base -1 (3 slots over j0+j3)
#   p=2: chi_4 base +1 (2 slots over j2)
#   p=3,4: constant +1 (1 slot)
for _p, (_a, _e) in enumerate([(6, -1.0), (9, -1.0), (4, 1.0), (0, 1.0), (0, 1.0)]):
    for _j in range(16):
        assert EPS1[_j, _p] == _e * ((-1.0) ** _popcount(_a & _j)), (
            f"EPS1 char mismatch p={_p}"
        )

# ----------------------------------------------------------------------------
# Geometry
# ----------------------------------------------------------------------------
N_TOTAL = 2097152
N_CORES = 8
NPC = N_TOTAL // N_CORES  # 262144 points per core
B = 128                   # points per j-block (free-dim inner run)
NJ = 16
MACRO = 128 * B           # 16384 points per macro tile
NM = NPC // MACRO         # 16 macros per core
UW = NJ * B               # 2048 u columns per macro
XTW = 50 * B              # xt blocks: grid-p0[16] grid-p1[16] grid-p2[16] hm hp
OW = 4 * B                # out channels: o0 o1 o2 s

WEIGHTS = {
    "wident": np.concatenate(
        [np.eye(128, dtype=np.float32), -np.eye(128, dtype=np.float32)], axis=1
    ).astype(ml_dtypes.bfloat16)
}


def _ap(t_ap, off, dims):
    """Custom free-dim AP on a tile: keep partition dim, replace free dims."""
    p = t_ap.ap[0]
    return AP(t_ap.tensor, t_ap.offset + off, [list(p)] + [list(d) for d in dims])


def _perm_ap(u_ap, c):
    """AP reading u[:, (j^c)*B + q] in plain (j,q) iteration order.
    Unflipped low j-bits merge into the innermost run, keeping every AP
    within the TENSOR3D 3-free-dim ISA limit."""
    if c == 0:
        return _ap(u_ap, 0, [[1, UW]])
    if c == 1:
        return _ap(u_ap, B, [[2 * B, 8], [-B, 2], [1, B]])
    if c == 2:
        return _ap(u_ap, 2 * B, [[4 * B, 4], [-2 * B, 2], [1, 2 * B]])
    if c == 4:
        return _ap(u_ap, 4 * B, [[8 * B, 2], [-4 * B, 2], [1, 4 * B]])
    if c == 8:
        return _ap(u_ap, 8 * B, [[-8 * B, 2], [1, 8 * B]])
    raise ValueError(c)


def build_bass():
    nc = bacc.Bacc("TRN2")

    u_d = nc.dram_tensor("u", [NM, 128, UW], BF16, kind="ExternalInput")
    xt_d = nc.dram_tensor("xt", [NM, 128, XTW], BF16, kind="ExternalInput")
    o_d = nc.dram_tensor("out", [NM, 128, OW], F32, kind="ExternalOutput")
    w_d = nc.dram_tensor("wident", [128, 256], BF16, kind="ExternalInput")

    from contextlib import ExitStack

    with tile.TileContext(nc) as tc, ExitStack() as ctx:
        wpool = ctx.enter_context(tc.tile_pool(name="wpool", bufs=1))
        w_sb = wpool.tile([128, 256], BF16, tag="wident")
        nc.sync.dma_start(w_sb[:], w_d[:])
        IP = w_sb[:, 0:128]   # +identity
        IN = w_sb[:, 128:256]  # -identity

        io_u = ctx.enter_context(tc.tile_pool(name="io_u", bufs=4))
        io_x = ctx.enter_context(tc.tile_pool(name="io_x", bufs=3))
        tp = ctx.enter_context(tc.tile_pool(name="tp", bufs=2))
        mxp = ctx.enter_context(tc.tile_pool(name="mxp", bufs=2))
        zp = ctx.enter_context(tc.tile_pool(name="zp", bufs=2))
        op = ctx.enter_context(tc.tile_pool(name="op", bufs=2))
        ps_mx = ctx.enter_context(tc.tile_pool(name="ps_mx", bufs=1, space="PSUM"))
        ps_o = ctx.enter_context(tc.tile_pool(name="ps_o", bufs=2, space="PSUM"))

        # per-macro state carried across the software pipeline
        state = {}  # m -> dict(u=..., mx=..., z=[...])

        def emit_fetch(m):
            u = io_u.tile([128, UW], BF16, tag="u")
            nc.sync.dma_start(u[:], u_d[m])
            xt = io_x.tile([128, XTW], BF16, tag="xt")
            nc.sync.dma_start(xt[:], xt_d[m])
            state[m] = {"u": u, "xt": xt}

        def emit_front(m, prev):
            """Stage-1 products (interleaved with prev's stage-2 DVE
            products) + PE accumulation + mx evac."""
            u, xt = state[m]["u"], state[m]["xt"]

            # stage-1 products on DVE: one op per channel.  p0-p2 read
            # full-grid sign-expanded x buffers (16 blocks, content
            # EPS1[j,p]*x_p); hm/hp are plain stride-0 broadcasts.
            ts = []
            spec = [
                (0, _ap(xt[:], 0, [[B, 16], [1, B]])),            # p0: c=0
                (1, _ap(xt[:], 16 * B, [[B, 16], [1, B]])),       # p1: c=1
                (2, _ap(xt[:], 32 * B, [[B, 16], [1, B]])),       # p2: c=2
                (4, _ap(xt[:], 48 * B, [[0, 16], [1, B]])),       # p3: c=4 (hm)
                (8, _ap(xt[:], 49 * B, [[0, 16], [1, B]])),       # p4: c=8 (hp)
            ]
            zq = (4, 1, 2, 3, 0)  # z4 first (s-tree), z0 last
            for i, (c, bc) in enumerate(spec):
                t = tp.tile([128, UW], BF16, tag=f"t{i}")
                nc.vector.tensor_mul(_ap(t[:], 0, [[1, UW]]), _perm_ap(u[:], c), bc)
                ts.append(t)
                # spread prev's stage-2 DVE products between stage-1 products
                if prev is not None:
                    emit_mid_dve(prev, q=zq[i])

            # PE: accumulate the five t tiles into PSUM f32 (p-major so the
            # accumulation chases the DVE product stream)
            mx_ps = ps_mx.tile([128, UW], F32, tag="mx_ps")
            for p in range(5):
                for b in range(4):
                    sl = slice(b * 512, (b + 1) * 512)
                    nc.tensor.matmul(
                        mx_ps[:, sl], IP, ts[p][:, sl], start=(p == 0), stop=(p == 4)
                    )

            mx = mxp.tile([128, UW], BF16, tag="mx")
            nc.scalar.copy(mx[:], mx_ps[:])
            state[m]["mx"] = mx

        def emit_mid_dve(m, q):
            """One stage-2 product on DVE.  All ten products run on DVE:
            co-running GpSimd drops concurrent DVE tensor ops out of 2x mode
            (global SBUF port contention, measured 1135ns -> 4990ns), so the
            Pool engine stays idle on purpose."""
            st = state[m]
            z = zp.tile([128, UW], BF16, tag=f"z{q}")
            nc.vector.tensor_mul(z[:], _perm_ap(st["u"][:], CS[q]), st["mx"][:])
            st.setdefault("z", {})[q] = z

        def emit_back(m):
            """PE contraction trees + out evac + DMA out."""
            zs = state[m]["z"]
            o_ps = ps_o.tile([128, OW], F32, tag="o_ps")
            # channels o1,o2 first (their z's finish earliest on DVE), then
            # s (z4 is Pool's first product), then o0 (z0 is Pool's second)
            for q in (1, 2):
                for j in range(16):
                    wgt = IP if W2[j, q] > 0 else IN
                    nc.tensor.matmul(
                        o_ps[:, q * B:(q + 1) * B],
                        wgt,
                        zs[q][:, j * B:(j + 1) * B],
                        start=(j == 0),
                        stop=(j == 15),
                    )
            # channel s = sum_j W2[j,4]*z4[j]  -  sum_j W2[j,3]*z3[j]
            for k, (q, flip) in enumerate(((4, 1.0), (3, -1.0))):
                for j in range(16):
                    wgt = IP if flip * W2[j, q] > 0 else IN
                    nc.tensor.matmul(
                        o_ps[:, 3 * B:4 * B],
                        wgt,
                        zs[q][:, j * B:(j + 1) * B],
                        start=(k == 0 and j == 0),
                        stop=(k == 1 and j == 15),
                    )
            for j in range(16):
                wgt = IP if W2[j, 0] > 0 else IN
                nc.tensor.matmul(
                    o_ps[:, 0:B],
                    wgt,
                    zs[0][:, j * B:(j + 1) * B],
                    start=(j == 0),
                    stop=(j == 15),
                )
            o_sb = op.tile([128, OW], F32, tag="o_sb")
            nc.scalar.copy(o_sb[:], o_ps[:])
            nc.sync.dma_start(o_d[m], o_sb[:])
            del state[m]

        # software pipeline: fetch(m+2) | front(m) [z(m-1) interleaved] |
        # back(m-1)
        emit_fetch(0)
        emit_fetch(1)
        emit_front(0, None)
        for m in range(1, NM):
            emit_fetch(m + 1) if m + 1 < NM else None
            emit_front(m, m - 1)
            emit_back(m - 1)
        for q in (4, 1, 2, 3, 0):
            emit_mid_dve(NM - 1, q)
        emit_back(NM - 1)

    nc.compile()
    return nc


_NC_CACHE = None


def _get_nc():
    global _NC_CACHE
    if _NC_CACHE is None:
        _NC_CACHE = build_bass()
    return _NC_CACHE


def _host_prep(versor, x):
    """Build the per-core input tensors (pure layout/sign/dtype transforms)."""
    # u[m,p,j*B+q] = TAU[j] * versor[n, j],  n = m*MACRO + p*B + q
    v5 = versor.reshape(N_CORES, NM, 128, B, 16)
    u = np.ascontiguousarray(
        np.transpose(v5, (0, 1, 2, 4, 3)) * TAU[None, None, None, :, None]
    ).astype(ml_dtypes.bfloat16)
    u = u.reshape(N_CORES, NM, 128, UW)

    xf = x.astype(np.float64)
    h = 0.5 * np.einsum("ij,ij->i", xf, xf)
    # 50 blocks: three full-grid channels (EPS1[j,p] * x_p for all 16 j),
    # then the two constant-sign h channels.
    blocks = [EPS1[j, p] * xf[:, p] for p in range(3) for j in range(16)]
    blocks += [h - 0.5, h + 0.5]
    xt = np.stack(blocks, axis=1)  # [N, 50]
    xt = xt.reshape(N_CORES, NM, 128, B, 50)
    xt = np.ascontiguousarray(np.transpose(xt, (0, 1, 2, 4, 3))).astype(
        ml_dtypes.bfloat16
    )
    xt = xt.reshape(N_CORES, NM, 128, XTW)
    return u, xt


def _in_maps(versor, x):
    u, xt = _host_prep(versor, x)
    in_maps = []
    for c in range(N_CORES):
        im = {"u": u[c], "xt": xt[c]}
        for name, arr in WEIGHTS.items():
            im[name] = arr
        in_maps.append(im)
    return in_maps


def _assemble(res):
    """Device [NM, 128, 4*B] channel tiles -> (N, 4) [num0,num1,num2,s]."""
    per_core = []
    for c in range(N_CORES):
        o = res.results[c]["out"].astype(np.float32).reshape(NM, 128, 4, B)
        per_core.append(np.transpose(o, (0, 1, 3, 2)).reshape(NPC, 4))
    return np.concatenate(per_core, axis=0)


def kernel(versor: np.ndarray, x: np.ndarray) -> np.ndarray:
    versor = np.ascontiguousarray(versor, dtype=np.float32)
    x = np.ascontiguousarray(x, dtype=np.float32)
    nc = _get_nc()
    res = run_bass_kernel_spmd(nc, _in_maps(versor, x), core_ids=list(range(N_CORES)))
    out4 = _assemble(res)
    num = out4[:, :3]
    sk = out4[:, 3]
    out = num / sk[:, None]

    # Conditioning fixup: bf16 on-chip products round at ~2^-9; points with a
    # small denominator s or large h amplify that beyond the error budget.
    # Recompute those few points exactly on the host.
    h = 0.5 * np.einsum("ij,ij->i", x, x)
    flag = (np.abs(sk) < 0.7) | (h > 4.5) | (np.abs(num).max(axis=1) > 4.0)
    if np.any(flag):
        out[flag] = _exact_ref(versor[flag], x[flag])
    return out.astype(np.float32)


def _exact_ref(versor, x):
    v = versor.astype(np.float64)
    xf = x.astype(np.float64)
    h = 0.5 * np.sum(xf * xf, axis=1)

    def X(c):
        return v[:, np.arange(16) ^ c]

    T0 = X(0) * (_s1[None, :, 0] * xf[:, 0:1])
    T1 = X(1) * (_s1[None, :, 1] * xf[:, 1:2])
    T2 = X(2) * (_s1[None, :, 2] * xf[:, 2:3])
    Vinf = _s1[None, :, 3] * X(4) + _s1[None, :, 4] * X(8)
    Cp = -0.5 * _s1[None, :, 3] * X(4) + 0.5 * _s1[None, :, 4] * X(8)
    mx = T0 + T1 + T2 + Vinf * h[:, None] + Cp
    D = _s2[None, :, 4] * X(8) - _s2[None, :, 3] * X(4)
    s = np.sum(mx * D, axis=1)
    num = np.stack(
        [np.sum(_s2[None, :, r] * (mx * X(r)), axis=1) for r in range(3)], axis=1
    )
    return (num / s[:, None]).astype(np.float32)


if __name__ == "__main__":
    rng = np.random.default_rng(0)
    v = (0.1 * rng.standard_normal((N_TOTAL, 16))).astype(np.float32)
    v[:, 0] += 1.0
    x = rng.standard_normal((N_TOTAL, 3)).astype(np.float32)
    out = kernel(versor=v, x=x)
    print("kernel ran, out shape", out.shape, out.dtype)


# revision 31
# speedup vs baseline: 1.6931x; 1.0650x over previous
"""Trainium2 Bass kernel for the CGA sandwich pipeline (nn_CGAPipeline).

out = decode( (V * encode(x)) * ~V ) over N=2^21 points, data-parallel over
8 NeuronCores.

v3 design ("POP" = point-on-partition layout, multi-engine roofline):

The v2 comp-major design was bound by PSUM-evacuation copies (ACT), 1x-mode
DVE products reading f32 PSUM, and slow gpsimd adds; all four engines sat at
50-80% of a 484us span.  v3 keeps every per-point tensor in a point-major
"comp-blocked" SBUF layout [128 part = point-rows, free = j*128 + q] where
j = odd-blade rank (16) and q = point-in-row (128):

- The five XOR-translation permutations j -> j^c of the versor become pure
  access patterns (multi-dim APs with negative strides), zero compute.
- The Clifford sign cocycle is split as s(j,p) = sigma(j)*tau(j^c)*chi(j):
  tau is folded into the host-shipped versor copy, sigma into the stage-2
  tree weights, and the residual characters chi into sign-alternating
  broadcast buffers (stage 1) and +-identity matmul weights (stage 2).
- Stage-1/stage-2 products are bf16 tensor_tensor ops in DVE 2x_1p mode
  (all-SBUF, unit innermost stride), split 8/2 between DVE and GpSimd.
- The j-sums (stage-1 term accumulation and stage-2 contraction trees) run
  on the otherwise-idle PE as +-identity matmuls accumulating in PSUM f32.
- ACT only evacuates mx and the 4 output channels; decode division and the
  ill-conditioned-point fixup stay on the host as in v2.
"""
import sys

sys.path.insert(0, "/opt/trn_rl_repo")

import ml_dtypes
import numpy as np

import concourse.bacc as bacc
import concourse.bass as bass
import concourse.mybir as mybir
import concourse.tile as tile
from concourse.bass_types import AP
from concourse.bass_utils import run_bass_kernel_spmd

F32 = mybir.dt.float32
BF16 = mybir.dt.bfloat16

# ----------------------------------------------------------------------------
# Cl(4,1) sign tables (rank-indexed; see reference.py for the blade algebra)
# ----------------------------------------------------------------------------
_METRIC = [1.0, 1.0, 1.0, 1.0, -1.0]


def _popcount(x):
    return bin(x).count("1")


def _blade_mul(a, b):
    s = 0
    t = a >> 1
    while t:
        s += _popcount(t & b)
        t >>= 1
    sign = -1.0 if (s & 1) else 1.0
    for i in range(5):
        if (a >> i) & 1 and (b >> i) & 1:
            sign *= _METRIC[i]
    return a ^ b, sign


def _rev_sign(b):
    g = _popcount(b)
    return -1.0 if (g * (g - 1) // 2) % 2 else 1.0


def _E_code(i):
    return (i << 1) | (_popcount(i) & 1)


def _O_code(j):
    return (j << 1) | ((_popcount(j) + 1) & 1)


_KAPPAS = [1, 2, 4, 8, 16]
CS = [k >> 1 for k in _KAPPAS]  # XOR-translation constants [0,1,2,4,8]
J16 = np.arange(16)

_s1 = np.zeros((16, 5), np.float64)
_s2 = np.zeros((16, 5), np.float64)
for _p, _kp in enumerate(_KAPPAS):
    _c = _kp >> 1
    for _j in range(16):
        _code, _sg = _blade_mul(_E_code(_j ^ _c), _kp)
        assert _code == _O_code(_j)
        _s1[_j, _p] = _sg
for _q, _kq in enumerate(_KAPPAS):
    _c = _kq >> 1
    for _j in range(16):
        _code, _sg = _blade_mul(_O_code(_j), _E_code(_j ^ _c))
        assert _code == _kq
        _s2[_j, _q] = _sg * _rev_sign(_E_code(_j ^ _c))

# Sign separation: s1[j,p] = SIGMA[j]*TAU[j^c_p]*EPS1[j,p] with EPS1 a GF(2)
# character per column; s2[j,q]*SIGMA[j]*TAU[j^c_q] = W2[j,q] goes into the
# stage-2 tree weights.  (sigma/tau found by exhaustive search.)
SIGMA = np.array([-1, 1, 1, 1, 1, 1, -1, 1, 1, 1, -1, 1, -1, 1, 1, 1], np.float64)
TAU = np.array([1, 1, -1, 1, -1, 1, 1, 1, 1, -1, -1, -1, -1, -1, 1, -1], np.float64)

EPS1 = np.stack([SIGMA * _s1[:, p] * TAU[J16 ^ CS[p]] for p in range(5)], axis=1)
W2 = np.stack([_s2[:, q] * SIGMA * TAU[J16 ^ CS[q]] for q in range(5)], axis=1)

# stage-1 residual characters: support of chi per channel, verified below
#   p=0: chi_6 base -1 (3-slot alternating buffer over j1+j2)
#   p=1: chi_9 base -1 (3 slots over j0+j3)
#   p=2: chi_4 base +1 (2 slots over j2)
#   p=3,4: constant +1 (1 slot)
for _p, (_a, _e) in enumerate([(6, -1.0), (9, -1.0), (4, 1.0), (0, 1.0), (0, 1.0)]):
    for _j in range(16):
        assert EPS1[_j, _p] == _e * ((-1.0) ** _popcount(_a & _j)), (
            f"EPS1 char mismatch p={_p}"
        )

# ----------------------------------------------------------------------------
# Geometry
# ----------------------------------------------------------------------------
N_TOTAL = 2097152
N_CORES = 8
NPC = N_TOTAL // N_CORES  # 262144 points per core
B = 128                   # points per j-block (free-dim inner run)
NJ = 16
MACRO = 128 * B           # 16384 points per macro tile
NM = NPC // MACRO         # 16 macros per core
UW = NJ * B               # 2048 u columns per macro
XTW = 50 * B              # xt blocks: grid-p0[16] grid-p1[16] grid-p2[16] hm hp
OW = 4 * B                # out channels: o0 o1 o2 s

WEIGHTS = {
    "wident": np.concatenate(
        [np.eye(128, dtype=np.float32), -np.eye(128, dtype=np.float32)], axis=1
    ).astype(ml_dtypes.bfloat16)
}


def _ap(t_ap, off, dims):
    """Custom free-dim AP on a tile: keep partition dim, replace free dims."""
    p = t_ap.ap[0]
    return AP(t_ap.tensor, t_ap.offset + off, [list(p)] + [list(d) for d in dims])


def _perm_ap(u_ap, c):
    """AP reading u[:, (j^c)*B + q] in plain (j,q) iteration order.
    Unflipped low j-bits merge into the innermost run, keeping every AP
    within the TENSOR3D 3-free-dim ISA limit."""
    if c == 0:
        return _ap(u_ap, 0, [[1, UW]])
    if c == 1:
        return _ap(u_ap, B, [[2 * B, 8], [-B, 2], [1, B]])
    if c == 2:
        return _ap(u_ap, 2 * B, [[4 * B, 4], [-2 * B, 2], [1, 2 * B]])
    if c == 4:
        return _ap(u_ap, 4 * B, [[8 * B, 2], [-4 * B, 2], [1, 4 * B]])
    if c == 8:
        return _ap(u_ap, 8 * B, [[-8 * B, 2], [1, 8 * B]])
    raise ValueError(c)


def build_bass():
    nc = bacc.Bacc("TRN2")

    u_d = nc.dram_tensor("u", [NM, 128, UW], BF16, kind="ExternalInput")
    u12_d = nc.dram_tensor("u12", [NM, 128, UW], BF16, kind="ExternalInput")
    xt_d = nc.dram_tensor("xt", [NM, 128, XTW], BF16, kind="ExternalInput")
    o_d = nc.dram_tensor("out", [NM, 128, OW], F32, kind="ExternalOutput")
    w_d = nc.dram_tensor("wident", [128, 256], BF16, kind="ExternalInput")

    from contextlib import ExitStack

    with tile.TileContext(nc) as tc, ExitStack() as ctx:
        wpool = ctx.enter_context(tc.tile_pool(name="wpool", bufs=1))
        w_sb = wpool.tile([128, 256], BF16, tag="wident")
        nc.sync.dma_start(w_sb[:], w_d[:])
        IP = w_sb[:, 0:128]   # +identity
        IN = w_sb[:, 128:256]  # -identity

        io_u = ctx.enter_context(tc.tile_pool(name="io_u", bufs=4))
        io_x = ctx.enter_context(tc.tile_pool(name="io_x", bufs=3))
        tp = ctx.enter_context(tc.tile_pool(name="tp", bufs=2))
        mxp = ctx.enter_context(tc.tile_pool(name="mxp", bufs=2))
        zp = ctx.enter_context(tc.tile_pool(name="zp", bufs=2))
        op = ctx.enter_context(tc.tile_pool(name="op", bufs=2))
        ps_mx = ctx.enter_context(tc.tile_pool(name="ps_mx", bufs=1, space="PSUM"))
        ps_o = ctx.enter_context(tc.tile_pool(name="ps_o", bufs=2, space="PSUM"))
        ps_d = ctx.enter_context(tc.tile_pool(name="ps_d", bufs=1, space="PSUM"))
        dp = ctx.enter_context(tc.tile_pool(name="dp", bufs=2))

        # per-macro state carried across the software pipeline
        state = {}  # m -> dict(u=..., mx=..., z=[...])

        def emit_fetch(m):
            u = io_u.tile([128, UW], BF16, tag="u")
            nc.sync.dma_start(u[:], u_d[m])
            u12 = io_u.tile([128, UW], BF16, tag="u12")
            nc.sync.dma_start(u12[:], u12_d[m])
            xt = io_x.tile([128, XTW], BF16, tag="xt")
            nc.sync.dma_start(xt[:], xt_d[m])
            state[m] = {"u": u, "u12": u12, "xt": xt}

        def emit_front(m, prev):
            """Stage-1 products (interleaved with prev's stage-2 DVE
            products) + PE accumulation + mx evac."""
            u, xt = state[m]["u"], state[m]["xt"]

            # stage-1 products on DVE: one op per channel.  p0-p2 read
            # full-grid sign-expanded x buffers (16 blocks, content
            # EPS1[j,p]*x_p); hm/hp are plain stride-0 broadcasts.
            ts = []
            spec = [
                (0, _ap(xt[:], 0, [[B, 16], [1, B]])),            # p0: c=0
                (1, _ap(xt[:], 16 * B, [[B, 16], [1, B]])),       # p1: c=1
                (2, _ap(xt[:], 32 * B, [[B, 16], [1, B]])),       # p2: c=2
                (4, _ap(xt[:], 48 * B, [[0, 16], [1, B]])),       # p3: c=4 (hm)
                (8, _ap(xt[:], 49 * B, [[0, 16], [1, B]])),       # p4: c=8 (hp)
            ]
            # Two t-products go first so the earliest z of the previous macro
            # never stalls on its mx evacuation; then alternate.
            zq = (None, 1, 2, "s", 0)
            for i, (c, bc) in enumerate(spec):
                t = tp.tile([128, UW], BF16, tag=f"t{i}")
                nc.vector.tensor_mul(_ap(t[:], 0, [[1, UW]]), _perm_ap(u[:], c), bc)
                ts.append(t)
                # spread prev's stage-2 DVE products between stage-1 products
                if prev is not None and zq[i] is not None:
                    emit_mid_dve(prev, q=zq[i])

            # PE: accumulate the five t tiles into PSUM f32 (p-major so the
            # accumulation chases the DVE product stream)
            mx_ps = ps_mx.tile([128, UW], F32, tag="mx_ps")
            for p in range(5):
                for b in range(4):
                    sl = slice(b * 512, (b + 1) * 512)
                    nc.tensor.matmul(
                        mx_ps[:, sl], IP, ts[p][:, sl], start=(p == 0), stop=(p == 4)
                    )

            mx = mxp.tile([128, UW], BF16, tag="mx")
            nc.scalar.copy(mx[:], mx_ps[:])
            state[m]["mx"] = mx

        def emit_mid_dve(m, q):
            """One stage-2 product on DVE.  All products run on DVE:
            co-running GpSimd drops concurrent DVE tensor ops out of 2x mode
            (global SBUF port contention, measured 1135ns -> 4990ns), so the
            Pool engine stays idle on purpose.  q="s" multiplies mx by the
            PE-prebuilt d tensor (the whole s-channel contraction vector)."""
            st = state[m]
            z = zp.tile([128, UW], BF16, tag=f"z{q}")
            if q == "s":
                nc.vector.tensor_mul(z[:], st["d"][:], st["mx"][:])
            else:
                nc.vector.tensor_mul(z[:], _perm_ap(st["u"][:], CS[q]), st["mx"][:])
            st.setdefault("z", {})[q] = z

        def emit_dbuild(m):
            """PE builds d = perm8(u) - perm4(u12) (the s-channel contraction
            vector, all Clifford signs folded) via +-identity matmuls.  Bank
            slices of an XOR-translated j-range are plain contiguous slices,
            so each call's rhs is a simple [128,512] window.  Two PSUM banks,
            two sequential halves."""
            st = state[m]
            u, u12 = st["u"], st["u12"]
            d_sb = dp.tile([128, UW], BF16, tag="d_sb")
            for half in range(2):
                d_ps = ps_d.tile([128, 2 * 512], F32, tag="d_ps")
                for b2 in range(2):
                    b = half * 2 + b2
                    sl = slice(b2 * 512, (b2 + 1) * 512)
                    j0 = 4 * b
                    s8 = slice((j0 ^ 8) * B, ((j0 ^ 8) * B) + 512)
                    s4 = slice((j0 ^ 4) * B, ((j0 ^ 4) * B) + 512)
                    nc.tensor.matmul(d_ps[:, sl], IP, u[:, s8], start=True, stop=False)
                    nc.tensor.matmul(d_ps[:, sl], IN, u12[:, s4], start=False, stop=True)
                nc.scalar.copy(d_sb[:, half * 1024:(half + 1) * 1024], d_ps[:])
            st["d"] = d_sb

        def emit_back(m):
            """PE contraction trees + out evac + DMA out."""
            zs = state[m]["z"]
            o_ps = ps_o.tile([128, OW], F32, tag="o_ps")
            # channel order follows z production: z1, z2, zs ("s"), z0
            for ch, q in ((1, 1), (2, 2), (3, "s"), (0, 0)):
                for j in range(16):
                    wgt = IP if (q == "s" or W2[j, q] > 0) else IN
                    nc.tensor.matmul(
                        o_ps[:, ch * B:(ch + 1) * B],
                        wgt,
                        zs[q][:, j * B:(j + 1) * B],
                        start=(j == 0),
                        stop=(j == 15),
                    )
            o_sb = op.tile([128, OW], F32, tag="o_sb")
            nc.scalar.copy(o_sb[:], o_ps[:])
            nc.sync.dma_start(o_d[m], o_sb[:])
            del state[m]

        # software pipeline: fetch(m+2) | front(m) [z(m-1) interleaved] |
        # back(m-1)
        emit_fetch(0)
        emit_fetch(1)
        emit_front(0, None)
        emit_dbuild(0)
        for m in range(1, NM):
            emit_fetch(m + 1) if m + 1 < NM else None
            emit_front(m, m - 1)
            emit_back(m - 1)
            emit_dbuild(m)
        for q in (1, 2, "s", 0):
            emit_mid_dve(NM - 1, q)
        emit_back(NM - 1)

    nc.compile()
    return nc


_NC_CACHE = None


def _get_nc():
    global _NC_CACHE
    if _NC_CACHE is None:
        _NC_CACHE = build_bass()
    return _NC_CACHE


def _host_prep(versor, x):
    """Build the per-core input tensors (pure layout/sign/dtype transforms)."""
    # u[m,p,j*B+q] = TAU[j] * versor[n, j],  n = m*MACRO + p*B + q
    # u12 = chi_12-signed copy (feeds the PE-built s-channel d vector)
    chi12 = np.array([(-1.0) ** _popcount(12 & j) for j in range(16)])
    v5 = np.transpose(versor.reshape(N_CORES, NM, 128, B, 16), (0, 1, 2, 4, 3))
    vt = v5 * TAU[None, None, None, :, None]
    u = np.ascontiguousarray(vt).astype(ml_dtypes.bfloat16).reshape(N_CORES, NM, 128, UW)
    u12 = np.ascontiguousarray(vt * chi12[None, None, None, :, None]).astype(
        ml_dtypes.bfloat16
    ).reshape(N_CORES, NM, 128, UW)

    xf = x.astype(np.float64)
    h = 0.5 * np.einsum("ij,ij->i", xf, xf)
    # 50 blocks: three full-grid channels (EPS1[j,p] * x_p for all 16 j),
    # then the two constant-sign h channels.
    blocks = [EPS1[j, p] * xf[:, p] for p in range(3) for j in range(16)]
    blocks += [h - 0.5, h + 0.5]
    xt = np.stack(blocks, axis=1)  # [N, 50]
    xt = xt.reshape(N_CORES, NM, 128, B, 50)
    xt = np.ascontiguousarray(np.transpose(xt, (0, 1, 2, 4, 3))).astype(
        ml_dtypes.bfloat16
    )
    xt = xt.reshape(N_CORES, NM, 128, XTW)
    return u, u12, xt


def _in_maps(versor, x):
    u, u12, xt = _host_prep(versor, x)
    in_maps = []
    for c in range(N_CORES):
        im = {"u": u[c], "u12": u12[c], "xt": xt[c]}
        for name, arr in WEIGHTS.items():
            im[name] = arr
        in_maps.append(im)
    return in_maps


def _assemble(res):
    """Device [NM, 128, 4*B] channel tiles -> (N, 4) [num0,num1,num2,s]."""
    per_core = []
    for c in range(N_CORES):
        o = res.results[c]["out"].astype(np.float32).reshape(NM, 128, 4, B)
        per_core.append(np.transpose(o, (0, 1, 3, 2)).reshape(NPC, 4))
    return np.concatenate(per_core, axis=0)


def kernel(versor: np.ndarray, x: np.ndarray) -> np.ndarray:
    versor = np.ascontiguousarray(versor, dtype=np.float32)
    x = np.ascontiguousarray(x, dtype=np.float32)
    nc = _get_nc()
    res = run_bass_kernel_spmd(nc, _in_maps(versor, x), core_ids=list(range(N_CORES)))
    out4 = _assemble(res)
    num = out4[:, :3]
    sk = out4[:, 3]
    out = num / sk[:, None]

    # Conditioning fixup: bf16 on-chip products round at ~2^-9; points with a
    # small denominator s or large h amplify that beyond the error budget.
    # Recompute those few points exactly on the host.
    h = 0.5 * np.einsum("ij,ij->i", x, x)
    flag = (np.abs(sk) < 0.7) | (h > 4.5) | (np.abs(num).max(axis=1) > 4.0)
    if np.any(flag):
        out[flag] = _exact_ref(versor[flag], x[flag])
    return out.astype(np.float32)


def _exact_ref(versor, x):
    v = versor.astype(np.float64)
    xf = x.astype(np.float64)
    h = 0.5 * np.sum(xf * xf, axis=1)

    def X(c):
        return v[:, np.arange(16) ^ c]

    T0 = X(0) * (_s1[None, :, 0] * xf[:, 0:1])
    T1 = X(1) * (_s1[None, :, 1] * xf[:, 1:2])
    T2 = X(2) * (_s1[None, :, 2] * xf[:, 2:3])
    Vinf = _s1[None, :, 3] * X(4) + _s1[None, :, 4] * X(8)
    Cp = -0.5 * _s1[None, :, 3] * X(4) + 0.5 * _s1[None, :, 4] * X(8)
    mx = T0 + T1 + T2 + Vinf * h[:, None] + Cp
    D = _s2[None, :, 4] * X(8) - _s2[None, :, 3] * X(4)
    s = np.sum(mx * D, axis=1)
    num = np.stack(
        [np.sum(_s2[None, :, r] * (mx * X(r)), axis=1) for r in range(3)], axis=1
    )
    return (num / s[:, None]).astype(np.float32)


if __name__ == "__main__":
    rng = np.random.default_rng(0)
    v = (0.1 * rng.standard_normal((N_TOTAL, 16))).astype(np.float32)
    v[:, 0] += 1.0
    x = rng.standard_normal((N_TOTAL, 3)).astype(np.float32)
    out = kernel(versor=v, x=x)
    print("kernel ran, out shape", out.shape, out.dtype)


# revision 35
# speedup vs baseline: 1.7102x; 1.0101x over previous
"""Trainium2 Bass kernel for the CGA sandwich pipeline (nn_CGAPipeline).

out = decode( (V * encode(x)) * ~V ) over N=2^21 points, data-parallel over
8 NeuronCores.

v3 design ("POP" = point-on-partition layout, multi-engine roofline):

The v2 comp-major design was bound by PSUM-evacuation copies (ACT), 1x-mode
DVE products reading f32 PSUM, and slow gpsimd adds; all four engines sat at
50-80% of a 484us span.  v3 keeps every per-point tensor in a point-major
"comp-blocked" SBUF layout [128 part = point-rows, free = j*128 + q] where
j = odd-blade rank (16) and q = point-in-row (128):

- The five XOR-translation permutations j -> j^c of the versor become pure
  access patterns (multi-dim APs with negative strides), zero compute.
- The Clifford sign cocycle is split as s(j,p) = sigma(j)*tau(j^c)*chi(j):
  tau is folded into the host-shipped versor copy, sigma into the stage-2
  tree weights, and the residual characters chi into sign-alternating
  broadcast buffers (stage 1) and +-identity matmul weights (stage 2).
- Stage-1/stage-2 products are bf16 tensor_tensor ops in DVE 2x_1p mode
  (all-SBUF, unit innermost stride), split 8/2 between DVE and GpSimd.
- The j-sums (stage-1 term accumulation and stage-2 contraction trees) run
  on the otherwise-idle PE as +-identity matmuls accumulating in PSUM f32.
- ACT only evacuates mx and the 4 output channels; decode division and the
  ill-conditioned-point fixup stay on the host as in v2.
"""
import sys

sys.path.insert(0, "/opt/trn_rl_repo")

import ml_dtypes
import numpy as np

import concourse.bacc as bacc
import concourse.bass as bass
import concourse.mybir as mybir
import concourse.tile as tile
from concourse.bass_types import AP
from concourse.bass_utils import run_bass_kernel_spmd

F32 = mybir.dt.float32
BF16 = mybir.dt.bfloat16

# ----------------------------------------------------------------------------
# Cl(4,1) sign tables (rank-indexed; see reference.py for the blade algebra)
# ----------------------------------------------------------------------------
_METRIC = [1.0, 1.0, 1.0, 1.0, -1.0]


def _popcount(x):
    return bin(x).count("1")


def _blade_mul(a, b):
    s = 0
    t = a >> 1
    while t:
        s += _popcount(t & b)
        t >>= 1
    sign = -1.0 if (s & 1) else 1.0
    for i in range(5):
        if (a >> i) & 1 and (b >> i) & 1:
            sign *= _METRIC[i]
    return a ^ b, sign


def _rev_sign(b):
    g = _popcount(b)
    return -1.0 if (g * (g - 1) // 2) % 2 else 1.0


def _E_code(i):
    return (i << 1) | (_popcount(i) & 1)


def _O_code(j):
    return (j << 1) | ((_popcount(j) + 1) & 1)


_KAPPAS = [1, 2, 4, 8, 16]
CS = [k >> 1 for k in _KAPPAS]  # XOR-translation constants [0,1,2,4,8]
J16 = np.arange(16)

_s1 = np.zeros((16, 5), np.float64)
_s2 = np.zeros((16, 5), np.float64)
for _p, _kp in enumerate(_KAPPAS):
    _c = _kp >> 1
    for _j in range(16):
        _code, _sg = _blade_mul(_E_code(_j ^ _c), _kp)
        assert _code == _O_code(_j)
        _s1[_j, _p] = _sg
for _q, _kq in enumerate(_KAPPAS):
    _c = _kq >> 1
    for _j in range(16):
        _code, _sg = _blade_mul(_O_code(_j), _E_code(_j ^ _c))
        assert _code == _kq
        _s2[_j, _q] = _sg * _rev_sign(_E_code(_j ^ _c))

# Sign separation: s1[j,p] = SIGMA[j]*TAU[j^c_p]*EPS1[j,p] with EPS1 a GF(2)
# character per column; s2[j,q]*SIGMA[j]*TAU[j^c_q] = W2[j,q] goes into the
# stage-2 tree weights.  (sigma/tau found by exhaustive search.)
SIGMA = np.array([-1, 1, 1, 1, 1, 1, -1, 1, 1, 1, -1, 1, -1, 1, 1, 1], np.float64)
TAU = np.array([1, 1, -1, 1, -1, 1, 1, 1, 1, -1, -1, -1, -1, -1, 1, -1], np.float64)

EPS1 = np.stack([SIGMA * _s1[:, p] * TAU[J16 ^ CS[p]] for p in range(5)], axis=1)
W2 = np.stack([_s2[:, q] * SIGMA * TAU[J16 ^ CS[q]] for q in range(5)], axis=1)

# stage-1 residual characters: support of chi per channel, verified below
#   p=0: chi_6 base -1 (3-slot alternating buffer over j1+j2)
#   p=1: chi_9 base -1 (3 slots over j0+j3)
#   p=2: chi_4 base +1 (2 slots over j2)
#   p=3,4: constant +1 (1 slot)
for _p, (_a, _e) in enumerate([(6, -1.0), (9, -1.0), (4, 1.0), (0, 1.0), (0, 1.0)]):
    for _j in range(16):
        assert EPS1[_j, _p] == _e * ((-1.0) ** _popcount(_a & _j)), (
            f"EPS1 char mismatch p={_p}"
        )

# ----------------------------------------------------------------------------
# Geometry
# ----------------------------------------------------------------------------
N_TOTAL = 2097152
N_CORES = 8
NPC = N_TOTAL // N_CORES  # 262144 points per core
B = 128                   # points per j-block (free-dim inner run)
NJ = 16
MACRO = 128 * B           # 16384 points per macro tile
NM = NPC // MACRO         # 16 macros per core
UW = NJ * B               # 2048 u columns per macro
XTW = 50 * B              # xt blocks: grid-p0[16] grid-p1[16] grid-p2[16] hm hp
OW = 4 * B                # out channels: o0 o1 o2 s

WEIGHTS = {
    "wident": np.concatenate(
        [np.eye(128, dtype=np.float32), -np.eye(128, dtype=np.float32)], axis=1
    ).astype(ml_dtypes.bfloat16)
}


def _ap(t_ap, off, dims):
    """Custom free-dim AP on a tile: keep partition dim, replace free dims."""
    p = t_ap.ap[0]
    return AP(t_ap.tensor, t_ap.offset + off, [list(p)] + [list(d) for d in dims])


def _perm_ap(u_ap, c):
    """AP reading u[:, (j^c)*B + q] in plain (j,q) iteration order.
    Unflipped low j-bits merge into the innermost run, keeping every AP
    within the TENSOR3D 3-free-dim ISA limit."""
    if c == 0:
        return _ap(u_ap, 0, [[1, UW]])
    if c == 1:
        return _ap(u_ap, B, [[2 * B, 8], [-B, 2], [1, B]])
    if c == 2:
        return _ap(u_ap, 2 * B, [[4 * B, 4], [-2 * B, 2], [1, 2 * B]])
    if c == 4:
        return _ap(u_ap, 4 * B, [[8 * B, 2], [-4 * B, 2], [1, 4 * B]])
    if c == 8:
        return _ap(u_ap, 8 * B, [[-8 * B, 2], [1, 8 * B]])
    raise ValueError(c)


def build_bass():
    nc = bacc.Bacc("TRN2")

    u_d = nc.dram_tensor("u", [NM, 128, UW], BF16, kind="ExternalInput")
    u12_d = nc.dram_tensor("u12", [NM, 128, UW], BF16, kind="ExternalInput")
    xt_d = nc.dram_tensor("xt", [NM, 128, XTW], BF16, kind="ExternalInput")
    o_d = nc.dram_tensor("out", [NM, 128, OW], F32, kind="ExternalOutput")
    w_d = nc.dram_tensor("wident", [128, 256], BF16, kind="ExternalInput")

    from contextlib import ExitStack

    with tile.TileContext(nc) as tc, ExitStack() as ctx:
        wpool = ctx.enter_context(tc.tile_pool(name="wpool", bufs=1))
        w_sb = wpool.tile([128, 256], BF16, tag="wident")
        nc.sync.dma_start(w_sb[:], w_d[:])
        IP = w_sb[:, 0:128]   # +identity
        IN = w_sb[:, 128:256]  # -identity

        io_u = ctx.enter_context(tc.tile_pool(name="io_u", bufs=4))
        io_x = ctx.enter_context(tc.tile_pool(name="io_x", bufs=3))
        tp = ctx.enter_context(tc.tile_pool(name="tp", bufs=2))
        mxp = ctx.enter_context(tc.tile_pool(name="mxp", bufs=2))
        zp = ctx.enter_context(tc.tile_pool(name="zp", bufs=2))
        op = ctx.enter_context(tc.tile_pool(name="op", bufs=2))
        ps_mx = ctx.enter_context(tc.tile_pool(name="ps_mx", bufs=1, space="PSUM"))
        ps_o = ctx.enter_context(tc.tile_pool(name="ps_o", bufs=2, space="PSUM"))
        ps_d = ctx.enter_context(tc.tile_pool(name="ps_d", bufs=1, space="PSUM"))
        dp = ctx.enter_context(tc.tile_pool(name="dp", bufs=2))

        # per-macro state carried across the software pipeline
        state = {}  # m -> dict(u=..., mx=..., z=[...])

        def emit_fetch(m, split=False):
            u = io_u.tile([128, UW], BF16, tag="u")
            nc.sync.dma_start(u[:], u_d[m])
            xt = io_x.tile([128, XTW], BF16, tag="xt")
            if split:
                # macro 0 fill: land each product's blocks in consumption
                # order so t0 starts ~8us earlier
                for lo, hi in ((0, 16), (16, 32), (32, 50)):
                    nc.sync.dma_start(xt[:, lo * B:hi * B], xt_d[m, :, lo * B:hi * B])
            else:
                nc.sync.dma_start(xt[:], xt_d[m])
            u12 = io_u.tile([128, UW], BF16, tag="u12")
            nc.sync.dma_start(u12[:], u12_d[m])
            state[m] = {"u": u, "u12": u12, "xt": xt}

        def emit_front(m, prev):
            """Stage-1 products (interleaved with prev's stage-2 DVE
            products) + PE accumulation + mx evac."""
            u, xt = state[m]["u"], state[m]["xt"]

            # stage-1 products on DVE: one op per channel.  p0-p2 read
            # full-grid sign-expanded x buffers (16 blocks, content
            # EPS1[j,p]*x_p); hm/hp are plain stride-0 broadcasts.
            ts = []
            spec = [
                (0, _ap(xt[:], 0, [[B, 16], [1, B]])),            # p0: c=0
                (1, _ap(xt[:], 16 * B, [[B, 16], [1, B]])),       # p1: c=1
                (2, _ap(xt[:], 32 * B, [[B, 16], [1, B]])),       # p2: c=2
                (4, _ap(xt[:], 48 * B, [[0, 16], [1, B]])),       # p3: c=4 (hm)
                (8, _ap(xt[:], 49 * B, [[0, 16], [1, B]])),       # p4: c=8 (hp)
            ]
            # Two t-products go first so the earliest z of the previous macro
            # never stalls on its mx evacuation; then alternate.
            zq = (None, 1, 2, "s", 0)
            for i, (c, bc) in enumerate(spec):
                t = tp.tile([128, UW], BF16, tag=f"t{i}")
                nc.vector.tensor_mul(_ap(t[:], 0, [[1, UW]]), _perm_ap(u[:], c), bc)
                ts.append(t)
                # spread prev's stage-2 DVE products between stage-1 products
                if prev is not None and zq[i] is not None:
                    emit_mid_dve(prev, q=zq[i])

            # PE: accumulate the five t tiles into PSUM f32 (p-major so the
            # accumulation chases the DVE product stream)
            mx_ps = ps_mx.tile([128, UW], F32, tag="mx_ps")
            for p in range(5):
                for b in range(4):
                    sl = slice(b * 512, (b + 1) * 512)
                    nc.tensor.matmul(
                        mx_ps[:, sl], IP, ts[p][:, sl], start=(p == 0), stop=(p == 4)
                    )

            mx = mxp.tile([128, UW], BF16, tag="mx")
            nc.scalar.copy(mx[:], mx_ps[:])
            state[m]["mx"] = mx

        def emit_mid_dve(m, q):
            """One stage-2 product on DVE.  All products run on DVE:
            co-running GpSimd drops concurrent DVE tensor ops out of 2x mode
            (global SBUF port contention, measured 1135ns -> 4990ns), so the
            Pool engine stays idle on purpose.  q="s" multiplies mx by the
            PE-prebuilt d tensor (the whole s-channel contraction vector)."""
            st = state[m]
            z = zp.tile([128, UW], BF16, tag=f"z{q}")
            if q == "s":
                nc.vector.tensor_mul(z[:], st["d"][:], st["mx"][:])
            else:
                nc.vector.tensor_mul(z[:], _perm_ap(st["u"][:], CS[q]), st["mx"][:])
            st.setdefault("z", {})[q] = z

        def emit_dbuild(m):
            """PE builds d = perm8(u) - perm4(u12) (the s-channel contraction
            vector, all Clifford signs folded) via +-identity matmuls.  Bank
            slices of an XOR-translated j-range are plain contiguous slices,
            so each call's rhs is a simple [128,512] window.  Two PSUM banks,
            two sequential halves."""
            st = state[m]
            u, u12 = st["u"], st["u12"]
            d_sb = dp.tile([128, UW], BF16, tag="d_sb")
            for half in range(2):
                d_ps = ps_d.tile([128, 2 * 512], F32, tag="d_ps")
                for b2 in range(2):
                    b = half * 2 + b2
                    sl = slice(b2 * 512, (b2 + 1) * 512)
                    j0 = 4 * b
                    s8 = slice((j0 ^ 8) * B, ((j0 ^ 8) * B) + 512)
                    s4 = slice((j0 ^ 4) * B, ((j0 ^ 4) * B) + 512)
                    nc.tensor.matmul(d_ps[:, sl], IP, u[:, s8], start=True, stop=False)
                    nc.tensor.matmul(d_ps[:, sl], IN, u12[:, s4], start=False, stop=True)
                nc.scalar.copy(d_sb[:, half * 1024:(half + 1) * 1024], d_ps[:])
            st["d"] = d_sb

        def emit_tree(m, o_ps, ch, q):
            zs = state[m]["z"]
            for j in range(16):
                wgt = IP if (q == "s" or W2[j, q] > 0) else IN
                nc.tensor.matmul(
                    o_ps[:, ch * B:(ch + 1) * B],
                    wgt,
                    zs[q][:, j * B:(j + 1) * B],
                    start=(j == 0),
                    stop=(j == 15),
                )

        def emit_back(m, o_ps=None):
            """PE contraction trees + out evac + DMA out."""
            if o_ps is None:
                o_ps = ps_o.tile([128, OW], F32, tag="o_ps")
                # channel order follows z production: z1, z2, zs ("s"), z0
                for ch, q in ((1, 1), (2, 2), (3, "s"), (0, 0)):
                    emit_tree(m, o_ps, ch, q)
            o_sb = op.tile([128, OW], F32, tag="o_sb")
            nc.scalar.copy(o_sb[:], o_ps[:])
            nc.sync.dma_start(o_d[m], o_sb[:])
            del state[m]

        # software pipeline: fetch(m+2) | front(m) [z(m-1) interleaved] |
        # back(m-1)
        emit_fetch(0, split=True)
        emit_fetch(1)
        emit_front(0, None)
        emit_dbuild(0)
        for m in range(1, NM):
            emit_fetch(m + 1) if m + 1 < NM else None
            emit_front(m, m - 1)
            emit_back(m - 1)
            emit_dbuild(m)
        # tail: interleave the last macro's trees with its z products
        L = NM - 1
        o_ps = ps_o.tile([128, OW], F32, tag="o_ps")
        for ch, q in ((1, 1), (2, 2), (3, "s"), (0, 0)):
            emit_mid_dve(L, q)
            emit_tree(L, o_ps, ch, q)
        emit_back(L, o_ps=o_ps)

    nc.compile()
    return nc


_NC_CACHE = None


def _get_nc():
    global _NC_CACHE
    if _NC_CACHE is None:
        _NC_CACHE = build_bass()
    return _NC_CACHE


def _host_prep(versor, x):
    """Build the per-core input tensors (pure layout/sign/dtype transforms)."""
    # u[m,p,j*B+q] = TAU[j] * versor[n, j],  n = m*MACRO + p*B + q
    # u12 = chi_12-signed copy (feeds the PE-built s-channel d vector)
    chi12 = np.array([(-1.0) ** _popcount(12 & j) for j in range(16)])
    v5 = np.transpose(versor.reshape(N_CORES, NM, 128, B, 16), (0, 1, 2, 4, 3))
    vt = v5 * TAU[None, None, None, :, None]
    u = np.ascontiguousarray(vt).astype(ml_dtypes.bfloat16).reshape(N_CORES, NM, 128, UW)
    u12 = np.ascontiguousarray(vt * chi12[None, None, None, :, None]).astype(
        ml_dtypes.bfloat16
    ).reshape(N_CORES, NM, 128, UW)

    xf = x.astype(np.float64)
    h = 0.5 * np.einsum("ij,ij->i", xf, xf)
    # 50 blocks: three full-grid channels (EPS1[j,p] * x_p for all 16 j),
    # then the two constant-sign h channels.
    blocks = [EPS1[j, p] * xf[:, p] for p in range(3) for j in range(16)]
    blocks += [h - 0.5, h + 0.5]
    xt = np.stack(blocks, axis=1)  # [N, 50]
    xt = xt.reshape(N_CORES, NM, 128, B, 50)
    xt = np.ascontiguousarray(np.transpose(xt, (0, 1, 2, 4, 3))).astype(
        ml_dtypes.bfloat16
    )
    xt = xt.reshape(N_CORES, NM, 128, XTW)
    return u, u12, xt


def _in_maps(versor, x):
    u, u12, xt = _host_prep(versor, x)
    in_maps = []
    for c in range(N_CORES):
        im = {"u": u[c], "u12": u12[c], "xt": xt[c]}
        for name, arr in WEIGHTS.items():
            im[name] = arr
        in_maps.append(im)
    return in_maps


def _assemble(res):
    """Device [NM, 128, 4*B] channel tiles -> (N, 4) [num0,num1,num2,s]."""
    per_core = []
    for c in range(N_CORES):
        o = res.results[c]["out"].astype(np.float32).reshape(NM, 128, 4, B)
        per_core.append(np.transpose(o, (0, 1, 3, 2)).reshape(NPC, 4))
    return np.concatenate(per_core, axis=0)


def kernel(versor: np.ndarray, x: np.ndarray) -> np.ndarray:
    versor = np.ascontiguousarray(versor, dtype=np.float32)
    x = np.ascontiguousarray(x, dtype=np.float32)
    nc = _get_nc()
    res = run_bass_kernel_spmd(nc, _in_maps(versor, x), core_ids=list(range(N_CORES)))
    out4 = _assemble(res)
    num = out4[:, :3]
    sk = out4[:, 3]
    out = num / sk[:, None]

    # Conditioning fixup: bf16 on-chip products round at ~2^-9; points with a
    # small denominator s or large h amplify that beyond the error budget.
    # Recompute those few points exactly on the host.
    h = 0.5 * np.einsum("ij,ij->i", x, x)
    flag = (np.abs(sk) < 0.7) | (h > 4.5) | (np.abs(num).max(axis=1) > 4.0)
    if np.any(flag):
        out[flag] = _exact_ref(versor[flag], x[flag])
    return out.astype(np.float32)


def _exact_ref(versor, x):
    v = versor.astype(np.float64)
    xf = x.astype(np.float64)
    h = 0.5 * np.sum(xf * xf, axis=1)

    def X(c):
        return v[:, np.arange(16) ^ c]

    T0 = X(0) * (_s1[None, :, 0] * xf[:, 0:1])
    T1 = X(1) * (_s1[None, :, 1] * xf[:, 1:2])
    T2 = X(2) * (_s1[None, :, 2] * xf[:, 2:3])
    Vinf = _s1[None, :, 3] * X(4) + _s1[None, :, 4] * X(8)
    Cp = -0.5 * _s1[None, :, 3] * X(4) + 0.5 * _s1[None, :, 4] * X(8)
    mx = T0 + T1 + T2 + Vinf * h[:, None] + Cp
    D = _s2[None, :, 4] * X(8) - _s2[None, :, 3] * X(4)
    s = np.sum(mx * D, axis=1)
    num = np.stack(
        [np.sum(_s2[None, :, r] * (mx * X(r)), axis=1) for r in range(3)], axis=1
    )
    return (num / s[:, None]).astype(np.float32)


if __name__ == "__main__":
    rng = np.random.default_rng(0)
    v = (0.1 * rng.standard_normal((N_TOTAL, 16))).astype(np.float32)
    v[:, 0] += 1.0
    x = rng.standard_normal((N_TOTAL, 3)).astype(np.float32)
    out = kernel(versor=v, x=x)
    print("kernel ran, out shape", out.shape, out.dtype)
